# revision 43
# baseline (speedup 1.0000x reference)
"""Multi-head causal self-attention (B=4, T=1024, d_model=2048, 16 heads of 128)
for 8 Trainium2 NeuronCores.

Sharding: hybrid data x tensor parallel. Core c handles batch b = c//2 and
head group g = c%2 (8 heads per core). Each core computes q/k/v projections
for its 8 heads, causal flash-style attention, and the out-projection rows
for those heads, producing a partial [1024, 2048] output for its batch.
The host sums the two partials per batch and adds the output bias.

All on-device layouts are feature-major so no transposes are needed anywhere:
  - x is shipped pre-transposed per batch: xt [2048, 1024] (fp16)
  - q, k are produced feature-major [dh, T] per head; v token-major [T, dh]
  - scores are computed transposed: S^T[kv, q] = k_fm.T @ q_fm (lhsT=k, rhs=q)
  - softmax denominator via ones[128,128] matmul (partition reduction on PE),
    which also broadcasts the per-q sum to all 128 partitions
  - attention output accumulates as out^T[dh, q] = v_tm.T @ exp(S^T)
  - out^T is exactly the lhsT the out-projection needs

Startup is DMA-issue-rate limited (~0.65us per descriptor on the sync
engine), so block-0 q and k projections consume their (xt, w) chunks in
arrival order: the contraction loop is OUTERMOST over 8 live PSUM tiles
(all 8 banks, scoped pools), which keeps the PE gap-free from ~9us and
avoids re-arming the HAM throttle. Small/late tensors (biases, mask, wo)
are issued from the scalar engine's DMA ring to keep the sync ring free
for the critical x/weight stream.
"""

import numpy as np

B, T, C = 4, 1024, 2048
H = 16          # total heads
HL = 8          # heads per core (local)
HB = 4          # heads per block
DH = 128        # head dim
KC = C // 128   # contraction chunks (16)
P = 128
NCORES = 8

_cache = {}


def _build():
    import concourse.bacc as bacc
    import concourse.mybir as mybir
    import concourse.tile as tile

    F32 = mybir.dt.float32
    F16 = mybir.dt.float16
    AF = mybir.ActivationFunctionType
    ALU = mybir.AluOpType

    nc = bacc.Bacc("TRN2", target_bir_lowering=False, debug=False)

    xt_d = nc.dram_tensor("xt", (C, T), F16, kind="ExternalInput")
    wq_d = nc.dram_tensor("wq", (C, HL * DH), F16, kind="ExternalInput")
    wk_d = nc.dram_tensor("wk", (C, HL * DH), F16, kind="ExternalInput")
    wv_d = nc.dram_tensor("wv", (C, HL * DH), F16, kind="ExternalInput")
    wo_d = nc.dram_tensor("wo", (HL * DH, C), F16, kind="ExternalInput")
    bq_d = nc.dram_tensor("bq", (P, HL), F32, kind="ExternalInput")
    bk_d = nc.dram_tensor("bk", (P, HL), F32, kind="ExternalInput")
    bvb_d = nc.dram_tensor("bvb", (P, HL * DH), F16, kind="ExternalInput")
    mt_d = nc.dram_tensor("mt", (P, P), F16, kind="ExternalInput")
    id_d = nc.dram_tensor("idm", (P, P), F16, kind="ExternalInput")
    part_d = nc.dram_tensor("part", (T, C), F16, kind="ExternalOutput")

    BW = HB * DH  # head-block feature width (512)

    xt_v = xt_d.rearrange("(o p) t -> p o t", p=P)
    wq_v = wq_d.rearrange("(o p) m -> p o m", p=P)
    wk_v = wk_d.rearrange("(o p) m -> p o m", p=P)
    wv_v = wv_d.rearrange("(o p) m -> p o m", p=P)

    with tile.TileContext(nc) as tc:
        with (
            tc.tile_pool(name="res", bufs=1) as res,
            tc.tile_pool(name="wblk", bufs=1) as wblk,
            tc.tile_pool(name="qkv", bufs=2) as qkv,
            tc.tile_pool(name="wp", bufs=3) as wp,
        ):
            bq_sb = res.tile([P, HL], F32, tag="bq")
            bk_sb = res.tile([P, HL], F32, tag="bk")
            bvb_sb = res.tile([P, HL * DH], F16, tag="bvb")
            mt_sb = res.tile([P, P], F16, tag="mt")
            id_sb = res.tile([P, P], F16, tag="idm")

            ones_sb = res.tile([P, P], F16, tag="ones")
            nc.vector.memset(ones_sb[:], 1.0)

            xts = []
            for kc in range(KC):
                xt_sb = res.tile([P, T], F16, tag=f"xt{kc}", name=f"xt{kc}")
                xts.append(xt_sb)
            wts = {w: [None] * KC for w in ("wq", "wk", "wv")}

            def load_w(wname, wv_, kc, blk):
                lo = blk * BW
                wt = wblk.tile(
                    [P, BW], F16, tag=f"{wname}{kc}", name=f"{wname}{kc}_{blk}"
                )
                nc.sync.dma_start(wt[:], wv_[:, kc, lo : lo + BW])
                wts[wname][kc] = wt

            # ---- Block-0 DMA issue, in the order the PE consumes it ----
            for kc in range(KC):
                nc.sync.dma_start(xts[kc][:], xt_v[:, kc, :])
                load_w("wq", wq_v, kc, 0)
            # small resident tensors go on the scalar engine's DMA ring so
            # they don't delay the critical sync-ring weight stream
            nc.scalar.dma_start(bq_sb[:], bq_d[:])
            nc.scalar.dma_start(bk_sb[:], bk_d[:])
            for kc in range(KC):
                load_w("wk", wk_v, kc, 0)
            for kc in range(KC):
                load_w("wv", wv_v, kc, 0)
            # needed only from v-proj / attention on; on the SYNC ring behind
            # the whole critical q/k/v weight stream (scalar-ring DMAs all
            # hoist to t~7us and would steal bandwidth from the first
            # xt/wq pairs, delaying the PE's first real work)
            nc.sync.dma_start(bvb_sb[:], bvb_d[:])
            nc.sync.dma_start(mt_sb[:], mt_d[:])
            nc.sync.dma_start(id_sb[:], id_d[:])

            wo_sb = res.tile([P, HL, C], F16, tag="wo")
            oT = res.tile([P, HL, T], F16, tag="oT")

            with tc.tile_pool(name="warm", bufs=1, space="PSUM") as wpool:
                warm = wpool.tile([P, P], F32, tag="warm")
                for _ in range(34):
                    nc.tensor.matmul(
                        warm[:], ones_sb[:], ones_sb[:], start=True, stop=True
                    )

            def alloc_qkv():
                qf = qkv.tile([P, HB, T], F16, tag="qf", name="qf")
                kf = qkv.tile([P, HB, T], F16, tag="kf", name="kf")
                vt = qkv.tile([P, T // P, BW], F16, tag="vt", name="vt")
                return qf, kf, vt

            def proj_qk_streaming(dst, wname, bsb, blk, sp):
                """q/k projection with the contraction loop outermost: the 8
                output tiles (4 heads x 2 t-chunks) stay live in all 8 PSUM
                banks and each arriving (xt, w) chunk feeds 8 matmuls.
                The pool is shared across the q, k and v sub-phases with
                per-tile tags, so each successor tile only waits on ITS
                predecessor's bias-add (a pool release here would barrier
                every engine on the full drain, ~1.4us per boundary)."""
                pts = [
                    sp.tile([P, 512], F32, tag=f"p{i}", name=f"p{wname}{i}")
                    for i in range(8)
                ]
                for kc in range(KC):
                    for i in range(8):
                        h, t = i // 2, i % 2
                        nc.tensor.matmul(
                            pts[i][:],
                            wts[wname][kc][:, h * DH : (h + 1) * DH],
                            xts[kc][:, t * 512 : (t + 1) * 512],
                            start=(kc == 0),
                            stop=(kc == KC - 1),
                        )
                for i in range(8):
                    h, t = i // 2, i % 2
                    dst_ap = dst[:, h, t * 512 : (t + 1) * 512]
                    bias_ap = bsb[:, blk * HB + h : blk * HB + h + 1]
                    if i % 2 == 0:
                        nc.scalar.activation(
                            dst_ap, pts[i][:], AF.Identity, bias=bias_ap
                        )
                    else:
                        nc.vector.tensor_tensor(
                            dst_ap,
                            pts[i][:],
                            bias_ap.to_broadcast((P, 512)),
                            ALU.add,
                        )

            def proj_qk_chunk(dst, wname, bsb, blk, ps, h, t, bufs=3):
                pt = ps.tile([P, 512], F32, tag="mm", bufs=bufs)
                for kc in range(KC):
                    nc.tensor.matmul(
                        pt[:],
                        wts[wname][kc][:, h * DH : (h + 1) * DH],
                        xts[kc][:, t * 512 : (t + 1) * 512],
                        start=(kc == 0),
                        stop=(kc == KC - 1),
                    )
                nc.vector.tensor_tensor(
                    dst[:, h, t * 512 : (t + 1) * 512],
                    pt[:],
                    bsb[:, blk * HB + h : blk * HB + h + 1].to_broadcast((P, 512)),
                    ALU.add,
                )

            def proj_v_chunk(vt, blk, ps, m, bufs=3, tag="mm"):
                lo = blk * BW
                pt = ps.tile([P, 512], F32, tag=tag, bufs=bufs)
                for kc in range(KC):
                    nc.tensor.matmul(
                        pt[:],
                        xts[kc][:, m * P : (m + 1) * P],
                        wts["wv"][kc][:],
                        start=(kc == 0),
                        stop=(kc == KC - 1),
                    )
                nc.vector.tensor_tensor(
                    vt[:, m, :], pt[:], bvb_sb[:, lo : lo + BW], ALU.add
                )

            def attn_steps(blk, ps, qkvt, qc_outer=False):
                """Causal attention, two heads interleaved, as a GENERATOR
                yielding a label after each pipeline unit so the caller can
                interleave projection/out-projection matmul chunks between
                units.  The exp stream saturates the scalar engine
                (~21us/block), so attention never gets its own phase: its PE
                work rides inside a PE-heavy phase instead.
                Engine budget per unit:
                  - scores for two kv-chunks pack tightly into one 2-bank
                    PSUM tile so one exp instruction covers both (the scalar
                    engine's ~200ns/instruction overhead would otherwise add
                    ~25% to the exp stream);
                  - the causal mask is accumulated on the PE (maskT @ I with
                    start=False) instead of a DVE tensor_tensor;
                  - the softmax denominator comes from a DVE f16 running sum
                    of E (4x-rate SBUF op) + ONE ones@esum matmul per
                    (head, q-chunk), instead of a PE matmul per kv-chunk.
                PSUM: st tag 2 bufs x 2 banks + att 2 banks, leaving 2 banks
                for the interleaved projection chunks."""
                qf, kf, vt = qkvt
                if qc_outer:
                    order = [
                        (hp, qc)
                        for qc in range(T // 512)
                        for hp in range(HB // 2)
                    ]
                else:
                    order = [
                        (hp, qc)
                        for hp in range(HB // 2)
                        for qc in range(T // 512)
                    ]
                for hp, qc in order:
                    pair = (2 * hp, 2 * hp + 1)  # local head idx within block
                    if True:
                        jmax = (qc + 1) * 4
                        ngr = jmax // 2
                        att = {}
                        esum = {}
                        for l in pair:
                            att[l] = ps.tile(
                                [P, 512], F32, tag="att", bufs=2, name=f"att{l}"
                            )
                            esum[l] = wp.tile(
                                [P, 512], F16, tag="es", bufs=2, name=f"es{l}"
                            )

                        def bounds(j):
                            s = max(512 * qc, 128 * j)
                            return s, 512 * qc + 512 - s

                        sts = {}

                        def issue_group(l, g):
                            st = ps.tile(
                                [P, 1024], F32, tag="st", bufs=2, name=f"st{l}"
                            )
                            offs = []
                            off = 0
                            for j in (2 * g, 2 * g + 1):
                                s, n = bounds(j)
                                # keep each matmul's output inside one bank
                                if off % 512 and off % 512 + n > 512:
                                    off = (off // 512 + 1) * 512
                                diag = 128 * j >= 512 * qc
                                nc.tensor.matmul(
                                    st[:, off : off + n],
                                    kf[:, l, j * P : (j + 1) * P],
                                    qf[:, l, s : 512 * qc + 512],
                                    start=True,
                                    stop=not diag,
                                )
                                if diag:
                                    # st[:, off:off+P] += mask (mt.T == mask)
                                    nc.tensor.matmul(
                                        st[:, off : off + P],
                                        mt_sb[:],
                                        id_sb[:],
                                        start=False,
                                        stop=True,
                                    )
                                offs.append((j, off, n))
                                off += n
                            sts[(l, g)] = (st, offs, off)

                        def do_exp(l, g):
                            st, offs, width = sts.pop((l, g))
                            E = wp.tile([P, 1024], F16, tag="E", bufs=3)
                            nc.scalar.activation(E[:, :width], st[:, :width], AF.Exp)
                            sts[(l, g, "E")] = (E, offs)

                        def consume(l, g):
                            E, offs = sts.pop((l, g, "E"))
                            for j, off, n in offs:
                                c0 = max(0, 128 * j - 512 * qc)
                                nc.tensor.matmul(
                                    att[l][:, c0:],
                                    vt[:, j, l * DH : (l + 1) * DH],
                                    E[:, off : off + n],
                                    start=(j == 0),
                                    stop=(j == jmax - 1),
                                )
                                if j == 0:
                                    nc.vector.tensor_copy(
                                        esum[l][:], E[:, off : off + n]
                                    )
                                else:
                                    nc.vector.tensor_tensor(
                                        esum[l][:, c0:],
                                        esum[l][:, c0:],
                                        E[:, off : off + n],
                                        ALU.add,
                                    )

                        for l in pair:
                            issue_group(l, 0)
                        yield (blk, hp, qc, "pre0")
                        for l in pair:
                            do_exp(l, 0)
                            if ngr > 1:
                                issue_group(l, 1)
                        yield (blk, hp, qc, "pre1")
                        for g in range(ngr):
                            for l in pair:
                                consume(l, g)
                                if g + 1 < ngr:
                                    do_exp(l, g + 1)
                                if g + 2 < ngr:
                                    issue_group(l, g + 2)
                            yield (blk, hp, qc, g)
                        # denominator: one 2-bank st slot holds both heads'
                        # ones @ esum (partition-sum broadcast to 128 rows)
                        den = ps.tile([P, 1024], F32, tag="st", bufs=2, name="den")
                        for i, l in enumerate(pair):
                            nc.tensor.matmul(
                                den[:, i * 512 : (i + 1) * 512],
                                ones_sb[:],
                                esum[l][:],
                                start=True,
                                stop=True,
                            )
                        for i, l in enumerate(pair):
                            hh = blk * HB + l
                            rc = wp.tile([P, 512], F32, tag="rc")
                            nc.vector.reciprocal_approx_fast(
                                rc[:], den[:, i * 512 : (i + 1) * 512]
                            )
                            nc.vector.tensor_tensor(
                                oT[:, hh, qc * 512 : (qc + 1) * 512],
                                att[l][:],
                                rc[:],
                                ALU.mult,
                            )
                        yield (blk, hp, qc, "end")

            part_v = part_d.rearrange("(mo p) n -> p mo n", p=P)

            def outproj_part1(ps, m, n2, nh):
                """First nh heads of an out-proj tile; the accumulation group
                stays open until outproj_part2 adds the remaining heads."""
                pt = ps.tile([P, 512], F32, tag="mm", bufs=2, name=f"op{m}_{n2}")
                for h in range(nh):
                    nc.tensor.matmul(
                        pt[:],
                        oT[:, h, m * P : (m + 1) * P],
                        wo_sb[:, h, n2 * 512 : (n2 + 1) * 512],
                        start=(h == 0),
                        stop=False,
                    )
                return pt

            def outproj_part2(pt, m, n2, nh):
                for h in range(nh, HL):
                    nc.tensor.matmul(
                        pt[:],
                        oT[:, h, m * P : (m + 1) * P],
                        wo_sb[:, h, n2 * 512 : (n2 + 1) * 512],
                        start=False,
                        stop=(h == HL - 1),
                    )
                po = wp.tile([P, 512], F16, tag="po")
                nc.vector.tensor_copy(po[:], pt[:])
                nc.sync.dma_start(part_v[:, m, n2 * 512 : (n2 + 1) * 512], po[:])

            def outproj_chunk(ps, m, n2):
                pt = ps.tile([P, 512], F32, tag="mm", bufs=2)
                for h in range(HL):
                    nc.tensor.matmul(
                        pt[:],
                        oT[:, h, m * P : (m + 1) * P],
                        wo_sb[:, h, n2 * 512 : (n2 + 1) * 512],
                        start=(h == 0),
                        stop=(h == HL - 1),
                    )
                last = m == T // P - 1 and n2 == C // 512 - 1
                if last:
                    # split the final tile so its copy+DMA tail is short
                    for q in range(4):
                        po = wp.tile([P, 128], F16, tag="pol", bufs=2)
                        nc.vector.tensor_copy(po[:], pt[:, q * 128 : (q + 1) * 128])
                        nc.sync.dma_start(
                            part_v[
                                :, m, n2 * 512 + q * 128 : n2 * 512 + (q + 1) * 128
                            ],
                            po[:],
                        )
                else:
                    po = wp.tile([P, 512], F16, tag="po")
                    nc.vector.tensor_copy(po[:], pt[:])
                    nc.sync.dma_start(part_v[:, m, n2 * 512 : (n2 + 1) * 512], po[:])

            # ---- Block 0: q/k stream per-chunk; v once data resident ----
            qkvt0 = alloc_qkv()
            qkvt1 = alloc_qkv()
            with tc.tile_pool(name="ps0", bufs=1, space="PSUM") as ps:
                proj_qk_streaming(qkvt0[0], "wq", bq_sb, 0, ps)
                proj_qk_streaming(qkvt0[1], "wk", bk_sb, 0, ps)
                for m in range(T // P):
                    proj_v_chunk(qkvt0[2], 0, ps, m, bufs=1, tag=f"p{m}")
                # block-1 weights: sync ring is free from here (issue order:
                # after block-0 wv).  Grouped per weight so the WAR-blocked
                # wv transfers cannot wedge the ring in front of wq/wk.
                for kc in range(KC):
                    load_w("wq", wq_v, kc, 1)
                for kc in range(KC):
                    load_w("wk", wk_v, kc, 1)
                for kc in range(KC):
                    load_w("wv", wv_v, kc, 1)
                # out-proj weights, needed only in phase 3.  On the SYNC ring
                # after the block-1 weight stream: the scheduler keeps
                # DMA-vs-DMA queue order, so this 2MB transfer cannot jump
                # ahead of the startup-critical streams (it would if issued on
                # the scalar ring, whose DMAs all hoist to t~7us).
                nc.sync.dma_start(
                    wo_sb[:], wo_d.rearrange("(h p) n -> p h n", p=P)
                )
                # one block-1 q chunk emitted INSIDE this pool (reusing tag
                # p0): the pool-release barrier that follows then drains the
                # v bias-adds behind this chunk's 16 matmuls instead of
                # idling the PE; its own bias-add is split across scalar and
                # DVE so the barrier's residual wait is half an add
                pq10 = ps.tile([P, 512], F32, tag="p0", name="pq10")
                for kc in range(KC):
                    nc.tensor.matmul(
                        pq10[:],
                        wts["wq"][kc][:, 0:DH],
                        xts[kc][:, 0:512],
                        start=(kc == 0),
                        stop=(kc == KC - 1),
                    )
                nc.scalar.activation(
                    qkvt1[0][:, 0, 0:256],
                    pq10[:, 0:256],
                    AF.Identity,
                    bias=bq_sb[:, HB : HB + 1],
                )
                nc.vector.tensor_tensor(
                    qkvt1[0][:, 0, 256:512],
                    pq10[:, 256:512],
                    bq_sb[:, HB : HB + 1].to_broadcast((P, 256)),
                    ALU.add,
                )

            # ---- Merged phase A: block-1 projections ∥ block-0 attention ----
            # Attention saturates the scalar engine but leaves the PE mostly
            # idle, so its units are interleaved between block-1 projection
            # chunks (pure PE work with no data dependence on attention 0 —
            # qf/kf/vt are double-buffered per block).
            with tc.tile_pool(name="psm", bufs=1, space="PSUM") as ps:
                chunks = []
                for h in range(HB):
                    for t in range(T // 512):
                        if h == 0 and t == 0:
                            continue  # emitted inside the ps0 scope above
                        chunks.append(
                            (proj_qk_chunk, (qkvt1[0], "wq", bq_sb, 1, ps, h, t))
                        )
                # k heads 0,1 are needed by attention-1's first pair; heads
                # 2,3 are deferred to merged phase B as filler there
                for h in range(2):
                    for t in range(T // 512):
                        chunks.append(
                            (proj_qk_chunk, (qkvt1[1], "wk", bk_sb, 1, ps, h, t))
                        )
                for m in range(4):
                    chunks.append((proj_v_chunk, (qkvt1[2], 1, ps, m)))
                NU = 24  # units yielded per attention block
                ci = 0
                nspread = len(chunks) - 2  # hold 2 back to cover the drain
                for k, _lab in enumerate(attn_steps(0, ps, qkvt0)):
                    want = (k + 1) * nspread // NU
                    while ci < want:
                        f, args = chunks[ci]
                        f(*args, bufs=2)
                        ci += 1
                while ci < len(chunks):
                    f, args = chunks[ci]
                    f(*args, bufs=2)
                    ci += 1

                # ---- Merged phase B: out-projection ∥ block-1 attention ----
                # (same pool/tags as phase A: a pool boundary here would
                # barrier all engines on the full attention-0 drain)
                # Out-proj tiles need oT from ALL 8 local heads, so emission
                # is gated on attention-1 progress: rows m0-3 unlock once
                # pair (6,7) finishes its first q-chunk, the rest after
                # attention 1 completes.  The leftover block-1 v chunks
                # (m4-7, only needed by attention-1's second q-chunk) fill
                # the first units.
                # fillers for the qc0 sub-phases: k heads 2,3 FIRST (pair
                # (6,7)'s scores need them from unit ~6 — emitting them any
                # later would deadlock the in-order PE queue), then v m4-7
                # (needed only by the qc1 sub-phases)
                fillers = []
                for h in range(2, HB):
                    for t in range(T // 512):
                        fillers.append(
                            (proj_qk_chunk, (qkvt1[1], "wk", bk_sb, 1, ps, h, t))
                        )
                fillers += [
                    (proj_v_chunk, (qkvt1[2], 1, ps, m)) for m in range(4, 8)
                ]
                out_lo = [(m, n2) for m in range(4) for n2 in range(C // 512)]
                out_hi = [(m, n2) for m in range(4, 8) for n2 in range(C // 512)]
                p1q0_done = False
                complete_next = False
                pending = []
                for lab in attn_steps(1, ps, qkvt1, qc_outer=True):
                    blk_, hp_, qc_, tag_ = lab
                    if complete_next:
                        # one unit past pair-(6,7) qc0's end: its den ->
                        # recip -> mult chain drains on DVE behind the score
                        # matmuls just emitted, so these completions don't
                        # stall the PE on oT
                        for pt, m, n2 in pending:
                            outproj_part2(pt, m, n2, 6)
                        pending = []
                        complete_next = False
                        p1q0_done = True
                    if hp_ == 1 and qc_ == 0 and tag_ == "end":
                        complete_next = True
                        continue
                    if qc_ == 0:
                        if fillers:
                            f, args = fillers.pop(0)
                            f(*args, bufs=2)
                        elif (
                            hp_ == 1
                            and not p1q0_done
                            and len(pending) < 2
                            and out_lo
                        ):
                            # heads 0-5 of an out-proj tile are already
                            # available (attention-0 + pair (4,5))
                            m, n2 = out_lo.pop(0)
                            pending.append((outproj_part1(ps, m, n2, 6), m, n2))
                    elif p1q0_done:
                        # keep one m<4 tile for after the loop: it has no
                        # dependence on attention-1's final DVE chain and
                        # covers the first m>=4 tile's wait for it
                        if len(out_lo) > 1:
                            m, n2 = out_lo.pop(0)
                            outproj_chunk(ps, m, n2)
                for m, n2 in out_lo + out_hi:
                    outproj_chunk(ps, m, n2)

    nc.compile()
    return nc


def _prep_inputs(x, w_qkv, b_qkv, w_out):
    """Build the 8 per-core input maps (host-side shard + layout prep)."""
    f16 = np.float16
    scale = np.float32(1.0 / np.sqrt(DH))

    xt = [np.ascontiguousarray(x[b].T).astype(f16) for b in range(B)]

    # causal mask for a diagonal 128x128 block, shipped transposed: the
    # kernel accumulates it onto the scores via  maskT.T @ I  on the PE
    mask = np.where(
        np.arange(P)[None, :] >= np.arange(P)[:, None], 0.0, -30000.0
    ).astype(f16)
    mt = np.ascontiguousarray(mask.T)
    idm = np.eye(P, dtype=f16)

    per_g = []
    for g in range(2):
        lo, hi = g * HL * DH, (g + 1) * HL * DH
        wq = np.ascontiguousarray(w_qkv[:, lo:hi] * scale).astype(f16)
        wk = np.ascontiguousarray(w_qkv[:, C + lo : C + hi]).astype(f16)
        wv = np.ascontiguousarray(w_qkv[:, 2 * C + lo : 2 * C + hi]).astype(f16)
        wo = np.ascontiguousarray(w_out[lo:hi, :]).astype(f16)
        bq = (b_qkv[lo:hi] * scale).astype(np.float32).reshape(HL, P).T.copy()
        bk = b_qkv[C + lo : C + hi].astype(np.float32).reshape(HL, P).T.copy()
        bv = b_qkv[2 * C + lo : 2 * C + hi].astype(f16)
        bvb = np.ascontiguousarray(np.broadcast_to(bv[None, :], (P, HL * DH)))
        per_g.append(dict(wq=wq, wk=wk, wv=wv, wo=wo, bq=bq, bk=bk, bvb=bvb))

    in_maps = []
    for c in range(NCORES):
        b, g = c // 2, c % 2
        m = dict(per_g[g])
        m["xt"] = xt[b]
        m["mt"] = mt
        m["idm"] = idm
        in_maps.append(m)
    return in_maps


def run(x, w_qkv, b_qkv, w_out, b_out, trace=False, **trace_kwargs):
    from concourse.bass_utils import run_bass_kernel_spmd

    x = np.asarray(x, dtype=np.float32)
    w_qkv = np.asarray(w_qkv, dtype=np.float32)
    b_qkv = np.asarray(b_qkv, dtype=np.float32)
    w_out = np.asarray(w_out, dtype=np.float32)
    b_out = np.asarray(b_out, dtype=np.float32)

    if "nc" not in _cache:
        _cache["nc"] = _build()
    nc = _cache["nc"]

    in_maps = _prep_inputs(x, w_qkv, b_qkv, w_out)
    res = run_bass_kernel_spmd(
        nc, in_maps, core_ids=list(range(NCORES)), trace=trace, **trace_kwargs
    )

    out = np.empty((B, T, C), np.float32)
    for b in range(B):
        out[b] = res.results[2 * b]["part"].astype(np.float32) + res.results[
            2 * b + 1
        ]["part"].astype(np.float32)
    out += b_out
    return out, res


def kernel(x, w_qkv, b_qkv, w_out, b_out):
    out, _ = run(x, w_qkv, b_qkv, w_out, b_out)
    return out


# revision 47
# speedup vs baseline: 1.0238x; 1.0238x over previous
"""Multi-head causal self-attention (B=4, T=1024, d_model=2048, 16 heads of 128)
for 8 Trainium2 NeuronCores.

Sharding: hybrid data x tensor parallel. Core c handles batch b = c//2 and
head group g = c%2 (8 heads per core). Each core computes q/k/v projections
for its 8 heads, causal flash-style attention, and the out-projection rows
for those heads, producing a partial [1024, 2048] output for its batch.
The host sums the two partials per batch and adds the output bias.

All on-device layouts are feature-major so no transposes are needed anywhere:
  - x is shipped pre-transposed per batch: xt [2048, 1024] (fp16)
  - q, k are produced feature-major [dh, T] per head; v token-major [T, dh]
  - scores are computed transposed: S^T[kv, q] = k_fm.T @ q_fm (lhsT=k, rhs=q)
  - softmax denominator via ones[128,128] matmul (partition reduction on PE),
    which also broadcasts the per-q sum to all 128 partitions
  - attention output accumulates as out^T[dh, q] = v_tm.T @ exp(S^T)
  - out^T is exactly the lhsT the out-projection needs

Startup is DMA-issue-rate limited (~0.65us per descriptor on the sync
engine), so block-0 q and k projections consume their (xt, w) chunks in
arrival order: the contraction loop is OUTERMOST over 8 live PSUM tiles
(all 8 banks, scoped pools), which keeps the PE gap-free from ~9us and
avoids re-arming the HAM throttle. Small/late tensors (biases, mask, wo)
are issued from the scalar engine's DMA ring to keep the sync ring free
for the critical x/weight stream.
"""

import numpy as np

B, T, C = 4, 1024, 2048
H = 16          # total heads
HL = 8          # heads per core (local)
HB = 4          # heads per block
DH = 128        # head dim
KC = C // 128   # contraction chunks (16)
P = 128
NCORES = 8

_cache = {}


def _build():
    import concourse.bacc as bacc
    import concourse.mybir as mybir
    import concourse.tile as tile

    F32 = mybir.dt.float32
    F16 = mybir.dt.float16
    AF = mybir.ActivationFunctionType
    ALU = mybir.AluOpType

    nc = bacc.Bacc("TRN2", target_bir_lowering=False, debug=False)

    xt_d = nc.dram_tensor("xt", (C, T), F16, kind="ExternalInput")
    wq_d = nc.dram_tensor("wq", (C, HL * DH), F16, kind="ExternalInput")
    wk_d = nc.dram_tensor("wk", (C, HL * DH), F16, kind="ExternalInput")
    wv_d = nc.dram_tensor("wv", (C, HL * DH), F16, kind="ExternalInput")
    wo_d = nc.dram_tensor("wo", (HL * DH, C), F16, kind="ExternalInput")
    bq_d = nc.dram_tensor("bq", (P, HL), F32, kind="ExternalInput")
    bk_d = nc.dram_tensor("bk", (P, HL), F32, kind="ExternalInput")
    bvb_d = nc.dram_tensor("bvb", (P, HL * DH), F16, kind="ExternalInput")
    mt_d = nc.dram_tensor("mt", (P, P), F16, kind="ExternalInput")
    id_d = nc.dram_tensor("idm", (P, P), F16, kind="ExternalInput")
    part_d = nc.dram_tensor("part", (T, C), F16, kind="ExternalOutput")

    BW = HB * DH  # head-block feature width (512)

    xt_v = xt_d.rearrange("(o p) t -> p o t", p=P)
    wq_v = wq_d.rearrange("(o p) m -> p o m", p=P)
    wk_v = wk_d.rearrange("(o p) m -> p o m", p=P)
    wv_v = wv_d.rearrange("(o p) m -> p o m", p=P)

    with tile.TileContext(nc) as tc:
        with (
            tc.tile_pool(name="res", bufs=1) as res,
            tc.tile_pool(name="wblk", bufs=1) as wblk,
            tc.tile_pool(name="qkv", bufs=2) as qkv,
            tc.tile_pool(name="wp", bufs=3) as wp,
        ):
            bq_sb = res.tile([P, HL], F32, tag="bq")
            bk_sb = res.tile([P, HL], F32, tag="bk")
            bvb_sb = res.tile([P, HL * DH], F16, tag="bvb")
            mt_sb = res.tile([P, P], F16, tag="mt")
            id_sb = res.tile([P, P], F16, tag="idm")

            ones_sb = res.tile([P, P], F16, tag="ones")
            nc.vector.memset(ones_sb[:], 1.0)

            xts = []
            for kc in range(KC):
                xt_sb = res.tile([P, T], F16, tag=f"xt{kc}", name=f"xt{kc}")
                xts.append(xt_sb)
            wts = {w: [None] * KC for w in ("wq", "wk", "wv")}

            def load_w(wname, wv_, kc, blk):
                lo = blk * BW
                wt = wblk.tile(
                    [P, BW], F16, tag=f"{wname}{kc}", name=f"{wname}{kc}_{blk}"
                )
                nc.sync.dma_start(wt[:], wv_[:, kc, lo : lo + BW])
                wts[wname][kc] = wt

            # ---- Block-0 DMA issue, in the order the PE consumes it ----
            for kc in range(KC):
                nc.sync.dma_start(xts[kc][:], xt_v[:, kc, :])
                load_w("wq", wq_v, kc, 0)
            # small resident tensors go on the scalar engine's DMA ring so
            # they don't delay the critical sync-ring weight stream
            nc.scalar.dma_start(bq_sb[:], bq_d[:])
            nc.scalar.dma_start(bk_sb[:], bk_d[:])
            for kc in range(KC):
                load_w("wk", wk_v, kc, 0)
            for kc in range(KC):
                load_w("wv", wv_v, kc, 0)
            # needed only from v-proj / attention on; on the SYNC ring behind
            # the whole critical q/k/v weight stream (scalar-ring DMAs all
            # hoist to t~7us and would steal bandwidth from the first
            # xt/wq pairs, delaying the PE's first real work)
            nc.sync.dma_start(bvb_sb[:], bvb_d[:])
            nc.sync.dma_start(mt_sb[:], mt_d[:])
            nc.sync.dma_start(id_sb[:], id_d[:])

            wo_sb = res.tile([P, HL, C], F16, tag="wo")
            oT = res.tile([P, HL, T], F16, tag="oT")

            with tc.tile_pool(name="warm", bufs=1, space="PSUM") as wpool:
                warm = wpool.tile([P, P], F32, tag="warm")
                for _ in range(34):
                    nc.tensor.matmul(
                        warm[:], ones_sb[:], ones_sb[:], start=True, stop=True
                    )

            def alloc_qkv():
                qf = qkv.tile([P, HB, T], F16, tag="qf", name="qf")
                kf = qkv.tile([P, HB, T], F16, tag="kf", name="kf")
                vt = qkv.tile([P, T // P, BW], F16, tag="vt", name="vt")
                return qf, kf, vt

            def proj_qk_streaming(dst, wname, bsb, blk, sp):
                """q/k projection with the contraction loop outermost: the 8
                output tiles (4 heads x 2 t-chunks) stay live in all 8 PSUM
                banks and each arriving (xt, w) chunk feeds 8 matmuls.
                The pool is shared across the q, k and v sub-phases with
                per-tile tags, so each successor tile only waits on ITS
                predecessor's bias-add (a pool release here would barrier
                every engine on the full drain, ~1.4us per boundary)."""
                pts = [
                    sp.tile([P, 512], F32, tag=f"p{i}", name=f"p{wname}{i}")
                    for i in range(8)
                ]
                for kc in range(KC):
                    for i in range(8):
                        h, t = i // 2, i % 2
                        nc.tensor.matmul(
                            pts[i][:],
                            wts[wname][kc][:, h * DH : (h + 1) * DH],
                            xts[kc][:, t * 512 : (t + 1) * 512],
                            start=(kc == 0),
                            stop=(kc == KC - 1),
                        )
                for i in range(8):
                    h, t = i // 2, i % 2
                    dst_ap = dst[:, h, t * 512 : (t + 1) * 512]
                    bias_ap = bsb[:, blk * HB + h : blk * HB + h + 1]
                    if i % 2 == 0:
                        nc.scalar.activation(
                            dst_ap, pts[i][:], AF.Identity, bias=bias_ap
                        )
                    else:
                        nc.vector.tensor_tensor(
                            dst_ap,
                            pts[i][:],
                            bias_ap.to_broadcast((P, 512)),
                            ALU.add,
                        )

            def proj_qk_chunk(dst, wname, bsb, blk, ps, h, t, bufs=3):
                pt = ps.tile([P, 512], F32, tag="mm", bufs=bufs)
                for kc in range(KC):
                    nc.tensor.matmul(
                        pt[:],
                        wts[wname][kc][:, h * DH : (h + 1) * DH],
                        xts[kc][:, t * 512 : (t + 1) * 512],
                        start=(kc == 0),
                        stop=(kc == KC - 1),
                    )
                nc.vector.tensor_tensor(
                    dst[:, h, t * 512 : (t + 1) * 512],
                    pt[:],
                    bsb[:, blk * HB + h : blk * HB + h + 1].to_broadcast((P, 512)),
                    ALU.add,
                )

            def proj_v_chunk(vt, blk, ps, m, bufs=3, tag="mm"):
                lo = blk * BW
                pt = ps.tile([P, 512], F32, tag=tag, bufs=bufs)
                for kc in range(KC):
                    nc.tensor.matmul(
                        pt[:],
                        xts[kc][:, m * P : (m + 1) * P],
                        wts["wv"][kc][:],
                        start=(kc == 0),
                        stop=(kc == KC - 1),
                    )
                nc.vector.tensor_tensor(
                    vt[:, m, :], pt[:], bvb_sb[:, lo : lo + BW], ALU.add
                )

            def attn_steps(blk, ps, qkvt, qc_outer=False):
                """Causal attention, two heads interleaved, as a GENERATOR
                yielding a label after each pipeline unit so the caller can
                interleave projection/out-projection matmul chunks between
                units.  The exp stream saturates the scalar engine
                (~21us/block), so attention never gets its own phase: its PE
                work rides inside a PE-heavy phase instead.
                Engine budget per unit:
                  - scores for two kv-chunks pack tightly into one 2-bank
                    PSUM tile so one exp instruction covers both (the scalar
                    engine's ~200ns/instruction overhead would otherwise add
                    ~25% to the exp stream);
                  - the causal mask is accumulated on the PE (maskT @ I with
                    start=False) instead of a DVE tensor_tensor;
                  - the softmax denominator comes from a DVE f16 running sum
                    of E (4x-rate SBUF op) + ONE ones@esum matmul per
                    (head, q-chunk), instead of a PE matmul per kv-chunk.
                PSUM: st tag 2 bufs x 2 banks + att 2 banks, leaving 2 banks
                for the interleaved projection chunks."""
                qf, kf, vt = qkvt
                if qc_outer:
                    order = [
                        (hp, qc)
                        for qc in range(T // 512)
                        for hp in range(HB // 2)
                    ]
                else:
                    order = [
                        (hp, qc)
                        for hp in range(HB // 2)
                        for qc in range(T // 512)
                    ]
                for hp, qc in order:
                    pair = (2 * hp, 2 * hp + 1)  # local head idx within block
                    if True:
                        jmax = (qc + 1) * 4
                        ngr = jmax // 2
                        att = {}
                        esum = {}
                        for l in pair:
                            att[l] = ps.tile(
                                [P, 512], F32, tag="att", bufs=2, name=f"att{l}"
                            )
                            esum[l] = wp.tile(
                                [P, 512], F16, tag="es", bufs=2, name=f"es{l}"
                            )

                        def bounds(j):
                            s = max(512 * qc, 128 * j)
                            return s, 512 * qc + 512 - s

                        sts = {}

                        def issue_group(l, g):
                            st = ps.tile(
                                [P, 1024], F32, tag="st", bufs=2, name=f"st{l}"
                            )
                            offs = []
                            off = 0
                            for j in (2 * g, 2 * g + 1):
                                s, n = bounds(j)
                                # keep each matmul's output inside one bank
                                if off % 512 and off % 512 + n > 512:
                                    off = (off // 512 + 1) * 512
                                diag = 128 * j >= 512 * qc
                                nc.tensor.matmul(
                                    st[:, off : off + n],
                                    kf[:, l, j * P : (j + 1) * P],
                                    qf[:, l, s : 512 * qc + 512],
                                    start=True,
                                    stop=not diag,
                                )
                                if diag:
                                    # st[:, off:off+P] += mask (mt.T == mask)
                                    nc.tensor.matmul(
                                        st[:, off : off + P],
                                        mt_sb[:],
                                        id_sb[:],
                                        start=False,
                                        stop=True,
                                    )
                                offs.append((j, off, n))
                                off += n
                            sts[(l, g)] = (st, offs, off)

                        def do_exp(l, g):
                            st, offs, width = sts.pop((l, g))
                            E = wp.tile([P, 1024], F16, tag="E", bufs=3)
                            nc.scalar.activation(E[:, :width], st[:, :width], AF.Exp)
                            sts[(l, g, "E")] = (E, offs)

                        def consume(l, g):
                            E, offs = sts.pop((l, g, "E"))
                            for j, off, n in offs:
                                c0 = max(0, 128 * j - 512 * qc)
                                nc.tensor.matmul(
                                    att[l][:, c0:],
                                    vt[:, j, l * DH : (l + 1) * DH],
                                    E[:, off : off + n],
                                    start=(j == 0),
                                    stop=(j == jmax - 1),
                                )
                                if j == 0:
                                    nc.vector.tensor_copy(
                                        esum[l][:], E[:, off : off + n]
                                    )
                                else:
                                    nc.vector.tensor_tensor(
                                        esum[l][:, c0:],
                                        esum[l][:, c0:],
                                        E[:, off : off + n],
                                        ALU.add,
                                    )

                        for l in pair:
                            issue_group(l, 0)
                        yield (blk, hp, qc, "pre0")
                        for l in pair:
                            do_exp(l, 0)
                            if ngr > 1:
                                issue_group(l, 1)
                        yield (blk, hp, qc, "pre1")
                        for g in range(ngr):
                            for l in pair:
                                consume(l, g)
                                if g + 1 < ngr:
                                    do_exp(l, g + 1)
                                if g + 2 < ngr:
                                    issue_group(l, g + 2)
                            yield (blk, hp, qc, g)
                        # denominator: one 2-bank st slot holds both heads'
                        # ones @ esum (partition-sum broadcast to 128 rows)
                        den = ps.tile([P, 1024], F32, tag="st", bufs=2, name="den")
                        for i, l in enumerate(pair):
                            nc.tensor.matmul(
                                den[:, i * 512 : (i + 1) * 512],
                                ones_sb[:],
                                esum[l][:],
                                start=True,
                                stop=True,
                            )
                        for i, l in enumerate(pair):
                            hh = blk * HB + l
                            rc = wp.tile([P, 512], F32, tag="rc")
                            nc.vector.reciprocal_approx_fast(
                                rc[:], den[:, i * 512 : (i + 1) * 512]
                            )
                            nc.vector.tensor_tensor(
                                oT[:, hh, qc * 512 : (qc + 1) * 512],
                                att[l][:],
                                rc[:],
                                ALU.mult,
                            )
                        yield (blk, hp, qc, "end")

            part_v = part_d.rearrange("(mo p) n -> p mo n", p=P)

            def outproj_part1(ps, m, n2, nh):
                """First nh heads of an out-proj tile; the accumulation group
                stays open until outproj_part2 adds the remaining heads."""
                pt = ps.tile([P, 512], F32, tag="mm", bufs=2, name=f"op{m}_{n2}")
                for h in range(nh):
                    nc.tensor.matmul(
                        pt[:],
                        oT[:, h, m * P : (m + 1) * P],
                        wo_sb[:, h, n2 * 512 : (n2 + 1) * 512],
                        start=(h == 0),
                        stop=False,
                    )
                return pt

            def outproj_part2(pt, m, n2, nh):
                for h in range(nh, HL):
                    nc.tensor.matmul(
                        pt[:],
                        oT[:, h, m * P : (m + 1) * P],
                        wo_sb[:, h, n2 * 512 : (n2 + 1) * 512],
                        start=False,
                        stop=(h == HL - 1),
                    )
                po = wp.tile([P, 512], F16, tag="po")
                nc.vector.tensor_copy(po[:], pt[:])
                nc.sync.dma_start(part_v[:, m, n2 * 512 : (n2 + 1) * 512], po[:])

            def outproj_chunk(ps, m, n2):
                pt = ps.tile([P, 512], F32, tag="mm", bufs=2)
                for h in range(HL):
                    nc.tensor.matmul(
                        pt[:],
                        oT[:, h, m * P : (m + 1) * P],
                        wo_sb[:, h, n2 * 512 : (n2 + 1) * 512],
                        start=(h == 0),
                        stop=(h == HL - 1),
                    )
                last = m == T // P - 1 and n2 == C // 512 - 1
                if last:
                    # split the final tile so its copy+DMA tail is short
                    for q in range(2):
                        po = wp.tile([P, 256], F16, tag="pol", bufs=2)
                        nc.vector.tensor_copy(po[:], pt[:, q * 256 : (q + 1) * 256])
                        nc.sync.dma_start(
                            part_v[
                                :, m, n2 * 512 + q * 256 : n2 * 512 + (q + 1) * 256
                            ],
                            po[:],
                        )
                else:
                    po = wp.tile([P, 512], F16, tag="po")
                    nc.vector.tensor_copy(po[:], pt[:])
                    nc.sync.dma_start(part_v[:, m, n2 * 512 : (n2 + 1) * 512], po[:])

            # ---- Block 0: q/k stream per-chunk; v once data resident ----
            qkvt0 = alloc_qkv()
            with tc.tile_pool(name="ps0", bufs=1, space="PSUM") as ps:
                proj_qk_streaming(qkvt0[0], "wq", bq_sb, 0, ps)
                proj_qk_streaming(qkvt0[1], "wk", bk_sb, 0, ps)
                for m in range(T // P):
                    proj_v_chunk(qkvt0[2], 0, ps, m, bufs=1, tag=f"p{m}")
                # block-1 weights: sync ring is free from here (issue order:
                # after block-0 wv).  Grouped per weight so the WAR-blocked
                # wv transfers cannot wedge the ring in front of wq/wk.
                for kc in range(KC):
                    load_w("wq", wq_v, kc, 1)
                for kc in range(KC):
                    load_w("wk", wk_v, kc, 1)
                for kc in range(KC):
                    load_w("wv", wv_v, kc, 1)
                # out-proj weights, needed only in phase 3.  On the SYNC ring
                # after the block-1 weight stream: the scheduler keeps
                # DMA-vs-DMA queue order, so this 2MB transfer cannot jump
                # ahead of the startup-critical streams (it would if issued on
                # the scalar ring, whose DMAs all hoist to t~7us).
                nc.sync.dma_start(
                    wo_sb[:], wo_d.rearrange("(h p) n -> p h n", p=P)
                )

            # ---- Merged phase A: block-1 projections ∥ block-0 attention ----
            # Attention saturates the scalar engine but leaves the PE mostly
            # idle, so its units are interleaved between block-1 projection
            # chunks (pure PE work with no data dependence on attention 0 —
            # qf/kf/vt are double-buffered per block).
            qkvt1 = alloc_qkv()
            with tc.tile_pool(name="psm", bufs=1, space="PSUM") as ps:
                chunks = []
                for h in range(HB):
                    for t in range(T // 512):
                        chunks.append(
                            (proj_qk_chunk, (qkvt1[0], "wq", bq_sb, 1, ps, h, t))
                        )
                # k heads 0,1 are needed by attention-1's first pair; heads
                # 2,3 are deferred to merged phase B as filler there
                for h in range(2):
                    for t in range(T // 512):
                        chunks.append(
                            (proj_qk_chunk, (qkvt1[1], "wk", bk_sb, 1, ps, h, t))
                        )
                for m in range(4):
                    chunks.append((proj_v_chunk, (qkvt1[2], 1, ps, m)))
                NU = 24  # units yielded per attention block
                ci = 0
                nspread = len(chunks) - 2  # hold 2 back to cover the drain
                for k, _lab in enumerate(attn_steps(0, ps, qkvt0)):
                    want = (k + 1) * nspread // NU
                    while ci < want:
                        f, args = chunks[ci]
                        f(*args, bufs=2)
                        ci += 1
                while ci < len(chunks):
                    f, args = chunks[ci]
                    f(*args, bufs=2)
                    ci += 1

                # ---- Merged phase B: out-projection ∥ block-1 attention ----
                # (same pool/tags as phase A: a pool boundary here would
                # barrier all engines on the full attention-0 drain)
                # Out-proj tiles need oT from ALL 8 local heads, so emission
                # is gated on attention-1 progress: rows m0-3 unlock once
                # pair (6,7) finishes its first q-chunk, the rest after
                # attention 1 completes.  The leftover block-1 v chunks
                # (m4-7, only needed by attention-1's second q-chunk) fill
                # the first units.
                # fillers for the qc0 sub-phases: k heads 2,3 FIRST (pair
                # (6,7)'s scores need them from unit ~6 — emitting them any
                # later would deadlock the in-order PE queue), then v m4-7
                # (needed only by the qc1 sub-phases)
                fillers = []
                for h in range(2, HB):
                    for t in range(T // 512):
                        fillers.append(
                            (proj_qk_chunk, (qkvt1[1], "wk", bk_sb, 1, ps, h, t))
                        )
                fillers += [
                    (proj_v_chunk, (qkvt1[2], 1, ps, m)) for m in range(4, 8)
                ]
                out_lo = [(m, n2) for m in range(4) for n2 in range(C // 512)]
                out_hi = [(m, n2) for m in range(4, 8) for n2 in range(C // 512)]
                p1q0_done = False
                complete_next = False
                pending = []
                for lab in attn_steps(1, ps, qkvt1, qc_outer=True):
                    blk_, hp_, qc_, tag_ = lab
                    if complete_next:
                        # one unit past pair-(6,7) qc0's end: its den ->
                        # recip -> mult chain drains on DVE behind the score
                        # matmuls just emitted, so these completions don't
                        # stall the PE on oT
                        for pt, m, n2 in pending:
                            outproj_part2(pt, m, n2, 6)
                        pending = []
                        complete_next = False
                        p1q0_done = True
                    if hp_ == 1 and qc_ == 0 and tag_ == "end":
                        complete_next = True
                        continue
                    if qc_ == 0:
                        if fillers:
                            f, args = fillers.pop(0)
                            f(*args, bufs=2)
                        elif (
                            hp_ == 1
                            and not p1q0_done
                            and len(pending) < 2
                            and out_lo
                        ):
                            # heads 0-5 of an out-proj tile are already
                            # available (attention-0 + pair (4,5))
                            m, n2 = out_lo.pop(0)
                            pending.append((outproj_part1(ps, m, n2, 6), m, n2))
                    elif p1q0_done:
                        # keep one m<4 tile for after the loop: it has no
                        # dependence on attention-1's final DVE chain and
                        # covers the first m>=4 tile's wait for it
                        if len(out_lo) > 1:
                            m, n2 = out_lo.pop(0)
                            outproj_chunk(ps, m, n2)
                for m, n2 in out_lo + out_hi:
                    outproj_chunk(ps, m, n2)

    nc.compile()
    return nc


def _prep_inputs(x, w_qkv, b_qkv, w_out):
    """Build the 8 per-core input maps (host-side shard + layout prep)."""
    f16 = np.float16
    scale = np.float32(1.0 / np.sqrt(DH))

    xt = [np.ascontiguousarray(x[b].T).astype(f16) for b in range(B)]

    # causal mask for a diagonal 128x128 block, shipped transposed: the
    # kernel accumulates it onto the scores via  maskT.T @ I  on the PE
    mask = np.where(
        np.arange(P)[None, :] >= np.arange(P)[:, None], 0.0, -30000.0
    ).astype(f16)
    mt = np.ascontiguousarray(mask.T)
    idm = np.eye(P, dtype=f16)

    per_g = []
    for g in range(2):
        lo, hi = g * HL * DH, (g + 1) * HL * DH
        wq = np.ascontiguousarray(w_qkv[:, lo:hi] * scale).astype(f16)
        wk = np.ascontiguousarray(w_qkv[:, C + lo : C + hi]).astype(f16)
        wv = np.ascontiguousarray(w_qkv[:, 2 * C + lo : 2 * C + hi]).astype(f16)
        wo = np.ascontiguousarray(w_out[lo:hi, :]).astype(f16)
        bq = (b_qkv[lo:hi] * scale).astype(np.float32).reshape(HL, P).T.copy()
        bk = b_qkv[C + lo : C + hi].astype(np.float32).reshape(HL, P).T.copy()
        bv = b_qkv[2 * C + lo : 2 * C + hi].astype(f16)
        bvb = np.ascontiguousarray(np.broadcast_to(bv[None, :], (P, HL * DH)))
        per_g.append(dict(wq=wq, wk=wk, wv=wv, wo=wo, bq=bq, bk=bk, bvb=bvb))

    in_maps = []
    for c in range(NCORES):
        b, g = c // 2, c % 2
        m = dict(per_g[g])
        m["xt"] = xt[b]
        m["mt"] = mt
        m["idm"] = idm
        in_maps.append(m)
    return in_maps


def run(x, w_qkv, b_qkv, w_out, b_out, trace=False, **trace_kwargs):
    from concourse.bass_utils import run_bass_kernel_spmd

    x = np.asarray(x, dtype=np.float32)
    w_qkv = np.asarray(w_qkv, dtype=np.float32)
    b_qkv = np.asarray(b_qkv, dtype=np.float32)
    w_out = np.asarray(w_out, dtype=np.float32)
    b_out = np.asarray(b_out, dtype=np.float32)

    if "nc" not in _cache:
        _cache["nc"] = _build()
    nc = _cache["nc"]

    in_maps = _prep_inputs(x, w_qkv, b_qkv, w_out)
    res = run_bass_kernel_spmd(
        nc, in_maps, core_ids=list(range(NCORES)), trace=trace, **trace_kwargs
    )

    out = np.empty((B, T, C), np.float32)
    for b in range(B):
        out[b] = res.results[2 * b]["part"].astype(np.float32) + res.results[
            2 * b + 1
        ]["part"].astype(np.float32)
    out += b_out
    return out, res


def kernel(x, w_qkv, b_qkv, w_out, b_out):
    out, _ = run(x, w_qkv, b_qkv, w_out, b_out)
    return out
